# revision 3
# baseline (speedup 1.0000x reference)
"""Trainium2 Bass kernel for the controlled-unitary problem.

reference semantics (control=0, num_qubits=13, dim=8192):
    mask bit = 1 << 12, so columns/rows with that bit set are idx 4096..8191.
    out[:, c0] = state[:, c0]                       (control bit off: untouched)
    out[:, c1] = state[:, c1] @ target[c1, c1]      (controlled unitary)

Device work: complex [256,4096] @ [4096,4096] GEMM.
Sharding: output columns of the GEMM split 8 ways (each core gets a
[4096, 512] slab of the target block; every weight byte moves once).

Per-core kernel (v4):
  - Karatsuba split: t1 = ar.br, t2 = ai.bi, t3 = (ar+ai).(br+bi);
    C_r = t1 - t2, C_i = t3 - t1 - t2. One derived plane per operand.
  - All inputs fully resident in SBUF; every DMA issued upfront.
  - 3 DMA streams: SP ring carries b_r, ACT ring carries b_i, the
    SWDGE (gpsimd) queue carries both A planes (first 2 k-tiles of A
    ride the rings for a fast start).
  - Warmup matmuls on a gpsimd-memset scratch tile ramp the PE
    p-state from ~6us while input DMA streams.
  - Last chunk is m-major; w = t1s + t2 is precomputed so only ONE
    DVE op (out_i = t3 - w) remains after the final matmul.
  - fp16 outputs halve the store traffic.
"""

import os

import numpy as np

BATCH = 256
DIM = 8192
HALF = 4096
N_CORES = 8
NSH = HALF // N_CORES  # 512 output columns per core
KT = HALF // 128  # 32 k-tiles
MT = BATCH // 128  # 2 m-tiles

DT_NAME = os.environ.get("KERNEL_DT", "float16")
CHUNKS = [int(x) for x in os.environ.get(
    "KERNEL_CHUNKS", "1,1,2,4,8,8,8").split(",")]
assert sum(CHUNKS) == KT
NWARM = int(os.environ.get("KERNEL_NWARM", "4"))
# how many leading chunks of A ride the HWDGE rings (rest go SWDGE)
A_RING_CHUNKS = int(os.environ.get("KERNEL_A_RING_CHUNKS", "2"))
OUT_DT_NAME = os.environ.get("KERNEL_OUT_DT", "float16")

_CACHE = {}


def _np_dtype(dt_name):
    return np.float16 if dt_name == "float16" else np.float32


def _build(dt_name):
    import concourse.mybir as mybir
    import concourse.tile as tile
    from concourse import bacc

    DT = getattr(mybir.dt, dt_name)
    ODT = getattr(mybir.dt, OUT_DT_NAME)
    F32 = mybir.dt.float32

    nc = bacc.Bacc("TRN2", target_bir_lowering=False, debug=False,
                   num_devices=N_CORES)

    a_r = nc.dram_tensor("a_r", [128, KT, BATCH], DT, kind="ExternalInput")
    a_i = nc.dram_tensor("a_i", [128, KT, BATCH], DT, kind="ExternalInput")
    b_r = nc.dram_tensor("b_r", [128, KT, NSH], DT, kind="ExternalInput")
    b_i = nc.dram_tensor("b_i", [128, KT, NSH], DT, kind="ExternalInput")
    c_r = nc.dram_tensor("c_r", [BATCH, NSH], ODT, kind="ExternalOutput")
    c_i = nc.dram_tensor("c_i", [BATCH, NSH], ODT, kind="ExternalOutput")

    with tile.TileContext(nc) as tc:
        with (
            tc.tile_pool(name="sb", bufs=1) as sb,
            tc.tile_pool(name="ps", bufs=1, space="PSUM") as ps_pool,
        ):
            A_r = sb.tile([128, KT, BATCH], DT, name="A_r")
            A_i = sb.tile([128, KT, BATCH], DT, name="A_i")
            A_s = sb.tile([128, KT, BATCH], DT, name="A_s")
            B_r = sb.tile([128, KT, NSH], DT, name="B_r")
            B_i = sb.tile([128, KT, NSH], DT, name="B_i")
            B_s = sb.tile([128, KT, NSH], DT, name="B_s")
            warm_w = sb.tile([128, 128], DT, name="warm_w")
            warm_x = sb.tile([128, NSH], DT, name="warm_x")

            ps = {}
            for m in range(MT):
                for t in ("t1", "t2", "t3"):
                    ps[(m, t)] = ps_pool.tile([128, NSH], F32,
                                              name=f"ps_{m}_{t}")
            ps_warm = ps_pool.tile([128, NSH], F32, name="ps_warm")

            # PE warmup: scratch memsets on gpsimd (its framework queue
            # drains earliest), then matmuls to ramp the p-state.
            nc.gpsimd.memset(warm_w[:], 0.0)
            nc.gpsimd.memset(warm_x[:], 0.0)
            for _ in range(NWARM):
                nc.tensor.matmul(ps_warm[:], warm_w[:], warm_x[:],
                                 start=True, stop=True)

            # A planes after the first A_RING_CHUNKS chunks go on the
            # SWDGE queue, interleaved a_r/a_i per chunk.
            k0 = 0
            for ci, ch in enumerate(CHUNKS):
                ksl = slice(k0, k0 + ch)
                if ci >= A_RING_CHUNKS:
                    nc.gpsimd.dma_start(A_r[:, ksl], a_r[:, ksl])
                    nc.gpsimd.dma_start(A_i[:, ksl], a_i[:, ksl])
                k0 += ch

            k0 = 0
            n_chunks = len(CHUNKS)
            for ci, ch in enumerate(CHUNKS):
                ksl = slice(k0, k0 + ch)
                if ci < A_RING_CHUNKS:
                    nc.sync.dma_start(A_r[:, ksl], a_r[:, ksl])
                    nc.scalar.dma_start(A_i[:, ksl], a_i[:, ksl])
                nc.sync.dma_start(B_r[:, ksl], b_r[:, ksl])
                nc.scalar.dma_start(B_i[:, ksl], b_i[:, ksl])
                nc.vector.tensor_tensor(A_s[:, ksl], A_r[:, ksl],
                                        A_i[:, ksl], mybir.AluOpType.add)
                nc.vector.tensor_tensor(B_s[:, ksl], B_r[:, ksl],
                                        B_i[:, ksl], mybir.AluOpType.add)

                operands = {
                    "t1": (A_r, B_r),
                    "t2": (A_i, B_i),
                    "t3": (A_s, B_s),
                }
                last_chunk = ci == n_chunks - 1
                if not last_chunk:
                    # product-major: t1 needs only A_r + B_r, so the PE
                    # starts before A_i/B_i/prep land
                    for t in ("t1", "t2", "t3"):
                        lhs, rhs = operands[t]
                        for kk in range(ch):
                            k = k0 + kk
                            for m in range(MT):
                                msl = slice(m * 128, (m + 1) * 128)
                                nc.tensor.matmul(
                                    ps[(m, t)][:], lhs[:, k, msl],
                                    rhs[:, k, :], start=(k == 0),
                                    stop=False,
                                )
                else:
                    # m-major: finish all of m0 first so its epilogue
                    # overlaps m1's matmuls; per product, issue the
                    # epilogue ops that only need completed banks.
                    for m in range(MT):
                        msl = slice(m * 128, (m + 1) * 128)
                        t1s = sb.tile([128, NSH], F32, name=f"t1s{m}")
                        w = sb.tile([128, NSH], F32, name=f"w{m}")
                        out_r = sb.tile([128, NSH], ODT, name=f"out_r{m}")
                        out_i = sb.tile([128, NSH], ODT, name=f"out_i{m}")
                        for t in ("t1", "t2", "t3"):
                            lhs, rhs = operands[t]
                            for kk in range(ch):
                                k = k0 + kk
                                nc.tensor.matmul(
                                    ps[(m, t)][:], lhs[:, k, msl],
                                    rhs[:, k, :], start=(k == 0),
                                    stop=(kk == ch - 1),
                                )
                            if t == "t1":
                                nc.scalar.copy(t1s[:], ps[(m, "t1")][:])
                            elif t == "t2":
                                # C_r = t1 - t2; w = t1 + t2
                                nc.vector.tensor_tensor(
                                    out_r[:], t1s[:], ps[(m, "t2")][:],
                                    mybir.AluOpType.subtract)
                                nc.vector.tensor_tensor(
                                    w[:], t1s[:], ps[(m, "t2")][:],
                                    mybir.AluOpType.add)
                            else:
                                # C_i = t3 - t1 - t2 = t3 - w
                                nc.vector.tensor_tensor(
                                    out_i[:], ps[(m, "t3")][:], w[:],
                                    mybir.AluOpType.subtract)
                        nc.sync.dma_start(c_r[msl, :], out_r[:])
                        nc.scalar.dma_start(c_i[msl, :], out_i[:])
                k0 += ch

    nc.compile()
    return nc


def _get_nc(dt_name):
    if dt_name not in _CACHE:
        _CACHE[dt_name] = _build(dt_name)
    return _CACHE[dt_name]


def _pack_kxm(mat_t, np_dt):
    # mat_t: [4096, F] (k-major) -> [128, KT, F] with k = kt*128 + p
    f = mat_t.shape[1]
    return np.ascontiguousarray(
        mat_t.reshape(KT, 128, f).transpose(1, 0, 2).astype(np_dt)
    )


def run_device(A, B, dt_name=DT_NAME, trace=False):
    """A: [256, 4096] complex64, B: [4096, 4096] complex64.
    Returns C = A @ B as [256, 4096] complex64 plus the raw results."""
    from concourse import bass_utils

    nc = _get_nc(dt_name)
    np_dt = _np_dtype(dt_name)

    at = A.T  # [4096, 256]
    a_r = _pack_kxm(np.ascontiguousarray(at.real), np_dt)
    a_i = _pack_kxm(np.ascontiguousarray(at.imag), np_dt)
    br_full = B.real
    bi_full = B.imag

    in_maps = []
    for c in range(N_CORES):
        csl = slice(c * NSH, (c + 1) * NSH)
        in_maps.append({
            "a_r": a_r,
            "a_i": a_i,
            "b_r": _pack_kxm(np.ascontiguousarray(br_full[:, csl]), np_dt),
            "b_i": _pack_kxm(np.ascontiguousarray(bi_full[:, csl]), np_dt),
        })

    res = bass_utils.run_bass_kernel_spmd(
        nc, in_maps, core_ids=list(range(N_CORES)), trace=trace
    )

    out = np.empty((BATCH, HALF), dtype=np.complex64)
    for c in range(N_CORES):
        csl = slice(c * NSH, (c + 1) * NSH)
        out.real[:, csl] = res.results[c]["c_r"].astype(np.float32)
        out.imag[:, csl] = res.results[c]["c_i"].astype(np.float32)
    return out, res


def kernel(state, target_matrix, control, num_qubits):
    state = np.asarray(state)
    target_matrix = np.asarray(target_matrix)
    control = int(control)
    num_qubits = int(num_qubits)
    dim = 1 << num_qubits

    assert state.shape == (BATCH, DIM) and dim == DIM, (
        "kernel hardcoded for [256, 8192]"
    )

    mask = 1 << (num_qubits - control - 1)
    idx = np.arange(dim)
    c1 = idx[(idx & mask) != 0]  # columns with control bit set

    if control == 0:
        A = state[:, HALF:]
        B = target_matrix[HALF:, HALF:]
    else:
        A = state[:, c1]
        B = target_matrix[np.ix_(c1, c1)]
    A = np.ascontiguousarray(A, dtype=np.complex64)
    B = np.ascontiguousarray(B, dtype=np.complex64)

    C, _ = run_device(A, B)

    out = state.astype(np.complex64, copy=True)
    out[:, c1] = C
    return out


# revision 13
# speedup vs baseline: 1.1260x; 1.1260x over previous
"""Trainium2 Bass kernel for the controlled-unitary problem.

reference semantics (control=0, num_qubits=13, dim=8192):
    mask bit = 1 << 12, so columns/rows with that bit set are idx 4096..8191.
    out[:, c0] = state[:, c0]                       (control bit off: untouched)
    out[:, c1] = state[:, c1] @ target[c1, c1]      (controlled unitary)

Device work: complex [256,4096] @ [4096,4096] GEMM.
Sharding: output columns of the GEMM split 8 ways (each core gets a
[4096, 512] slab of the target block; every weight byte moves once).

Per-core kernel (v4):
  - Karatsuba split: t1 = ar.br, t2 = ai.bi, t3 = (ar+ai).(br+bi);
    C_r = t1 - t2, C_i = t3 - t1 - t2. One derived plane per operand.
  - All inputs fully resident in SBUF; every DMA issued upfront.
  - 3 DMA streams: SP ring carries b_r, ACT ring carries b_i, the
    SWDGE (gpsimd) queue carries both A planes (first 2 k-tiles of A
    ride the rings for a fast start).
  - Warmup matmuls on a gpsimd-memset scratch tile ramp the PE
    p-state from ~6us while input DMA streams.
  - Last chunk is m-major; w = t1s + t2 is precomputed so only ONE
    DVE op (out_i = t3 - w) remains after the final matmul.
  - fp16 outputs halve the store traffic.
"""

import os

import numpy as np

BATCH = 256
DIM = 8192
HALF = 4096
N_CORES = 8
NSH = HALF // N_CORES  # 512 output columns per core
KT = HALF // 128  # 32 k-tiles
MT = BATCH // 128  # 2 m-tiles

DT_NAME = os.environ.get("KERNEL_DT", "float16")
CHUNKS = [int(x) for x in os.environ.get(
    "KERNEL_CHUNKS", "1,1,2,2,4,4,6,6,6").split(",")]
assert sum(CHUNKS) == KT
NWARM = int(os.environ.get("KERNEL_NWARM", "2"))
# how many leading chunks of A ride the HWDGE rings (rest go SWDGE)
A_RING_CHUNKS = int(os.environ.get("KERNEL_A_RING_CHUNKS", "99"))
OUT_DT_NAME = os.environ.get("KERNEL_OUT_DT", "float16")
# split the last m-tile's t3 product into two N-halves so the first
# half of C_i stores while the second half's matmuls still run:
#   0 = off, 1 = separate half PSUM tiles for all chunks,
#   2 = full-N accumulation, split only inside the last chunk
NSPLIT = int(os.environ.get("KERNEL_NSPLIT", "0"))

_CACHE = {}


def _np_dtype(dt_name):
    return np.float16 if dt_name == "float16" else np.float32


def _build(dt_name):
    import concourse.mybir as mybir
    import concourse.tile as tile
    from concourse import bacc

    DT = getattr(mybir.dt, dt_name)
    ODT = getattr(mybir.dt, OUT_DT_NAME)
    F32 = mybir.dt.float32

    nc = bacc.Bacc("TRN2", target_bir_lowering=False, debug=False,
                   num_devices=N_CORES)

    a_r = nc.dram_tensor("a_r", [128, KT, BATCH], DT, kind="ExternalInput")
    a_i = nc.dram_tensor("a_i", [128, KT, BATCH], DT, kind="ExternalInput")
    b_r = nc.dram_tensor("b_r", [128, KT, NSH], DT, kind="ExternalInput")
    b_i = nc.dram_tensor("b_i", [128, KT, NSH], DT, kind="ExternalInput")
    c_r = nc.dram_tensor("c_r", [BATCH, NSH], ODT, kind="ExternalOutput")
    c_i = nc.dram_tensor("c_i", [BATCH, NSH], ODT, kind="ExternalOutput")

    with tile.TileContext(nc) as tc:
        with (
            tc.tile_pool(name="sb", bufs=1) as sb,
            tc.tile_pool(name="ps", bufs=1, space="PSUM") as ps_pool,
        ):
            A_r = sb.tile([128, KT, BATCH], DT, name="A_r")
            A_i = sb.tile([128, KT, BATCH], DT, name="A_i")
            A_s = sb.tile([128, KT, BATCH], DT, name="A_s")
            B_r = sb.tile([128, KT, NSH], DT, name="B_r")
            B_i = sb.tile([128, KT, NSH], DT, name="B_i")
            B_s = sb.tile([128, KT, NSH], DT, name="B_s")
            warm_w = sb.tile([128, 128], DT, name="warm_w")
            warm_x = sb.tile([128, NSH], DT, name="warm_x")

            ps = {}
            for m in range(MT):
                for t in ("t1", "t2", "t3"):
                    if NSPLIT == 1 and m == MT - 1 and t == "t3":
                        continue
                    ps[(m, t)] = ps_pool.tile([128, NSH], F32,
                                              name=f"ps_{m}_{t}")
            if NSPLIT == 1:
                HN = NSH // 2
                ps_t3h = [
                    ps_pool.tile([128, HN], F32, name=f"ps_t3h{h}")
                    for h in range(2)
                ]
            ps_warm = ps_pool.tile([128, NSH], F32, name="ps_warm")

            # PE warmup: scratch memsets on gpsimd (its framework queue
            # drains earliest), then matmuls to ramp the p-state.
            nc.gpsimd.memset(warm_w[:], 0.0)
            nc.gpsimd.memset(warm_x[:], 0.0)
            for _ in range(NWARM):
                nc.tensor.matmul(ps_warm[:], warm_w[:], warm_x[:],
                                 start=True, stop=True)

            # A planes after the first A_RING_CHUNKS chunks go on the
            # SWDGE queue, interleaved a_r/a_i per chunk.
            k0 = 0
            for ci, ch in enumerate(CHUNKS):
                ksl = slice(k0, k0 + ch)
                if ci >= A_RING_CHUNKS:
                    nc.gpsimd.dma_start(A_r[:, ksl], a_r[:, ksl])
                    nc.gpsimd.dma_start(A_i[:, ksl], a_i[:, ksl])
                k0 += ch

            k0 = 0
            n_chunks = len(CHUNKS)
            for ci, ch in enumerate(CHUNKS):
                ksl = slice(k0, k0 + ch)
                if ci < A_RING_CHUNKS:
                    nc.sync.dma_start(A_r[:, ksl], a_r[:, ksl])
                    nc.scalar.dma_start(A_i[:, ksl], a_i[:, ksl])
                nc.sync.dma_start(B_r[:, ksl], b_r[:, ksl])
                nc.scalar.dma_start(B_i[:, ksl], b_i[:, ksl])
                nc.vector.tensor_tensor(A_s[:, ksl], A_r[:, ksl],
                                        A_i[:, ksl], mybir.AluOpType.add)
                nc.vector.tensor_tensor(B_s[:, ksl], B_r[:, ksl],
                                        B_i[:, ksl], mybir.AluOpType.add)

                operands = {
                    "t1": (A_r, B_r),
                    "t2": (A_i, B_i),
                    "t3": (A_s, B_s),
                }
                last_chunk = ci == n_chunks - 1
                if not last_chunk:
                    # product-major: t1 needs only A_r + B_r, so the PE
                    # starts before A_i/B_i/prep land
                    for t in ("t1", "t2", "t3"):
                        lhs, rhs = operands[t]
                        for kk in range(ch):
                            k = k0 + kk
                            for m in range(MT):
                                msl = slice(m * 128, (m + 1) * 128)
                                if NSPLIT == 1 and m == MT - 1 and t == "t3":
                                    HN = NSH // 2
                                    for h in range(2):
                                        nsl = slice(h * HN, (h + 1) * HN)
                                        nc.tensor.matmul(
                                            ps_t3h[h][:],
                                            lhs[:, k, msl],
                                            rhs[:, k, nsl],
                                            start=(k == 0), stop=False,
                                        )
                                else:
                                    nc.tensor.matmul(
                                        ps[(m, t)][:], lhs[:, k, msl],
                                        rhs[:, k, :], start=(k == 0),
                                        stop=False,
                                    )
                else:
                    # m-major: finish all of m0 first so its epilogue
                    # overlaps m1's matmuls; per product, issue the
                    # epilogue ops that only need completed banks.
                    for m in range(MT):
                        msl = slice(m * 128, (m + 1) * 128)
                        t1s = sb.tile([128, NSH], F32, name=f"t1s{m}")
                        w = sb.tile([128, NSH], F32, name=f"w{m}")
                        out_r = sb.tile([128, NSH], ODT, name=f"out_r{m}")
                        out_i = sb.tile([128, NSH], ODT, name=f"out_i{m}")
                        split = NSPLIT != 0 and m == MT - 1
                        for t in ("t1", "t2", "t3"):
                            lhs, rhs = operands[t]
                            if t == "t3" and split:
                                HN = NSH // 2
                                for h in range(2):
                                    nsl = slice(h * HN, (h + 1) * HN)
                                    dst = (ps_t3h[h][:] if NSPLIT == 1
                                           else ps[(m, "t3")][:, nsl])
                                    for kk in range(ch):
                                        k = k0 + kk
                                        nc.tensor.matmul(
                                            dst,
                                            lhs[:, k, msl],
                                            rhs[:, k, nsl],
                                            start=(k == 0),
                                            stop=(kk == ch - 1),
                                            skip_group_check=True,
                                        )
                                    nc.vector.tensor_tensor(
                                        out_i[:, nsl], dst,
                                        w[:, nsl],
                                        mybir.AluOpType.subtract)
                                    nc.scalar.dma_start(
                                        c_i[msl, nsl], out_i[:, nsl])
                                continue
                            for kk in range(ch):
                                k = k0 + kk
                                nc.tensor.matmul(
                                    ps[(m, t)][:], lhs[:, k, msl],
                                    rhs[:, k, :], start=(k == 0),
                                    stop=(kk == ch - 1),
                                )
                            if t == "t1":
                                nc.scalar.copy(t1s[:], ps[(m, "t1")][:])
                            elif t == "t2":
                                # C_r = t1 - t2; w = t1 + t2
                                nc.vector.tensor_tensor(
                                    out_r[:], t1s[:], ps[(m, "t2")][:],
                                    mybir.AluOpType.subtract)
                                nc.sync.dma_start(c_r[msl, :], out_r[:])
                                nc.vector.tensor_tensor(
                                    w[:], t1s[:], ps[(m, "t2")][:],
                                    mybir.AluOpType.add)
                            else:
                                # C_i = t3 - t1 - t2 = t3 - w
                                nc.vector.tensor_tensor(
                                    out_i[:], ps[(m, "t3")][:], w[:],
                                    mybir.AluOpType.subtract)
                                nc.scalar.dma_start(
                                    c_i[msl, :], out_i[:])
                k0 += ch

    nc.compile()
    return nc


def _get_nc(dt_name):
    if dt_name not in _CACHE:
        _CACHE[dt_name] = _build(dt_name)
    return _CACHE[dt_name]


def _pack_kxm(mat_t, np_dt):
    # mat_t: [4096, F] (k-major) -> [128, KT, F] with k = kt*128 + p
    f = mat_t.shape[1]
    return np.ascontiguousarray(
        mat_t.reshape(KT, 128, f).transpose(1, 0, 2).astype(np_dt)
    )


def run_device(A, B, dt_name=DT_NAME, trace=False):
    """A: [256, 4096] complex64, B: [4096, 4096] complex64.
    Returns C = A @ B as [256, 4096] complex64 plus the raw results."""
    from concourse import bass_utils

    nc = _get_nc(dt_name)
    np_dt = _np_dtype(dt_name)

    at = A.T  # [4096, 256]
    a_r = _pack_kxm(np.ascontiguousarray(at.real), np_dt)
    a_i = _pack_kxm(np.ascontiguousarray(at.imag), np_dt)
    br_full = B.real
    bi_full = B.imag

    in_maps = []
    for c in range(N_CORES):
        csl = slice(c * NSH, (c + 1) * NSH)
        in_maps.append({
            "a_r": a_r,
            "a_i": a_i,
            "b_r": _pack_kxm(np.ascontiguousarray(br_full[:, csl]), np_dt),
            "b_i": _pack_kxm(np.ascontiguousarray(bi_full[:, csl]), np_dt),
        })

    res = bass_utils.run_bass_kernel_spmd(
        nc, in_maps, core_ids=list(range(N_CORES)), trace=trace
    )

    out = np.empty((BATCH, HALF), dtype=np.complex64)
    for c in range(N_CORES):
        csl = slice(c * NSH, (c + 1) * NSH)
        out.real[:, csl] = res.results[c]["c_r"].astype(np.float32)
        out.imag[:, csl] = res.results[c]["c_i"].astype(np.float32)
    return out, res


def kernel(state, target_matrix, control, num_qubits):
    state = np.asarray(state)
    target_matrix = np.asarray(target_matrix)
    control = int(control)
    num_qubits = int(num_qubits)
    dim = 1 << num_qubits

    assert state.shape == (BATCH, DIM) and dim == DIM, (
        "kernel hardcoded for [256, 8192]"
    )

    mask = 1 << (num_qubits - control - 1)
    idx = np.arange(dim)
    c1 = idx[(idx & mask) != 0]  # columns with control bit set

    if control == 0:
        A = state[:, HALF:]
        B = target_matrix[HALF:, HALF:]
    else:
        A = state[:, c1]
        B = target_matrix[np.ix_(c1, c1)]
    A = np.ascontiguousarray(A, dtype=np.complex64)
    B = np.ascontiguousarray(B, dtype=np.complex64)

    C, _ = run_device(A, B)

    out = state.astype(np.complex64, copy=True)
    out[:, c1] = C
    return out
